# revision 1
# baseline (speedup 1.0000x reference)
"""Cross-attention kernel for Trainium2 (8 NeuronCores, data-parallel over batch).

Problem (hardcoded): B=8, Sq=4096, Sk=77, E=1024, C=768, H=16 heads, D=64.

    q = x @ wq + bq; k = y @ wk + bk; v = y @ wv + bv
    out = softmax(q k^T / sqrt(D)) v @ wo + bo

Sharding: batch element b -> core b. No collectives.

Per-core device pipeline (all matmuls contract over the SBUF partition dim):
  - Activations are kept feature-major ("transposed"): xT[E, Sq] is prepared
    host-side, so QT = wq^T-free matmul chain produces qT[E, Sq] directly,
    per-head slices qT[h*64:(h+1)*64, :] feed scores without any on-chip
    transpose.
  - kT[E, Sk] via lhsT=wk_aug tiles; V[Sk, E] row-major via lhsT=yT_aug tiles.
    Biases for k/v are folded in by augmenting y with a ones-row (host side).
  - scores^T[Sk, q] = matmul(lhsT=kT head slice [64, 77], rhs=qT head slice).
    The 1/sqrt(D) scale is folded into wq/bq host-side.
  - softmax without max-subtraction (scores are O(5), exp is safe in fp32):
    exp on ScalarE; per-head denominators via one-hot [77, 16] matmuls
    accumulated into one PSUM tile; reciprocal on VectorE; broadcast across
    partitions via SBUF->SBUF DMA (stride-0 partition source).
  - o^T = matmul(lhsT=V head slice [77, 64], rhs=exps), normalized during
    PSUM eviction (DVE multiply by the broadcast reciprocal), written into
    oT[E, q] with even/odd heads at partition offsets 0/64 of paired tiles.
  - out[q, E] row-major = matmul(lhsT=oT tiles [128, 128], rhs=wo tiles),
    bias bo added during eviction from a partition-broadcast bias tile.

All matmul operands are typed float32r (fp32 with 11 mantissa bits): 1
cycle/row on the PE at N=512 (4x the plain-fp32 rate). Operands coming from
DRAM are pre-rounded host-side (round-to-nearest-even to the 20-bit format);
on-chip producers round by writing float32r-typed outputs.
"""

import os
from contextlib import ExitStack

import numpy as np

import concourse.bass as bass
import concourse.tile as tile
from concourse import bacc, mybir
from concourse.bass_utils import run_bass_kernel_spmd

N_CORES = 8
SQ = 4096
SK = 77
SKP = 80  # SK padded: fp32r matmul dst free-size must be even
E = 1024
C = 768
H = 16
D = 64
CHUNK = 512
NCHUNK = SQ // CHUNK  # 8
ET = E // 128  # 8 e-tiles
CT = C // 128  # 6 c-tiles
F32 = mybir.dt.float32
F32R = mybir.dt.float32r

_PROGRAM = None


def _round_f32r(a: np.ndarray) -> np.ndarray:
    """Round fp32 to the fp32r format (11 mantissa bits, RNE)."""
    u = np.ascontiguousarray(a, dtype=np.float32).view(np.uint32).copy()
    u += np.uint32(0x7FF) + ((u >> np.uint32(12)) & np.uint32(1))
    u &= np.uint32(0xFFFFF000)
    return u.view(np.float32)


def _build_program():
    nc = bacc.Bacc(
        "TRN2", target_bir_lowering=False, debug=False, num_devices=N_CORES
    )
    # xT pre-tiled host-side: [chunk, partition, e-tile, col] so each chunk's
    # SBUF tile is one contiguous 2MB DRAM read (16KB per partition row).
    xT_d = nc.dram_tensor(
        "xT", [NCHUNK, 128, ET, CHUNK], F32R, kind="ExternalInput"
    ).ap()
    yT_d = nc.dram_tensor("yT", [C, SKP], F32R, kind="ExternalInput").ap()
    wq_d = nc.dram_tensor("wq", [E, E], F32R, kind="ExternalInput").ap()
    bq_d = nc.dram_tensor("bq", [E], F32, kind="ExternalInput").ap()
    wk_d = nc.dram_tensor("wk", [C, E], F32R, kind="ExternalInput").ap()
    bk_d = nc.dram_tensor("bk", [E], F32, kind="ExternalInput").ap()
    wv_d = nc.dram_tensor("wv", [C, H * 64], F32R, kind="ExternalInput").ap()
    bv_d = nc.dram_tensor("bv", [H * 64], F32, kind="ExternalInput").ap()
    wo_d = nc.dram_tensor("wo", [E, E], F32R, kind="ExternalInput").ap()
    bo_d = nc.dram_tensor("bo", [E], F32, kind="ExternalInput").ap()
    oh_d = nc.dram_tensor("oh", [SK, H * H], F32R, kind="ExternalInput").ap()
    sel_d = nc.dram_tensor("sel", [H, ET * 128], F32R, kind="ExternalInput").ap()
    out_d = nc.dram_tensor("out", [SQ, E], F32, kind="ExternalOutput").ap()

    with tile.TileContext(nc) as tc, ExitStack() as ctx:
        consts = ctx.enter_context(tc.tile_pool(name="consts", bufs=1))
        wq_sb = consts.tile([128, ET, E], F32R)
        wo_sb = consts.tile([128, ET, E], F32R)
        kT_sb = consts.tile([128, ET, SKP], F32R)
        v_sb = consts.tile([SK, H * 64], F32R)
        oh_sb = consts.tile([SK, H * H], F32R)
        sel_sb = consts.tile([H, ET * 128], F32R)
        bq_sb = consts.tile([128, ET], F32)
        bk_sb = consts.tile([128, ET], F32)
        bv_sb = consts.tile([SK, H * 64], F32)
        bo_sb = consts.tile([128, E], F32)

        nc.sync.dma_start(oh_sb[:], oh_d)
        nc.sync.dma_start(sel_sb[:], sel_d)
        nc.sync.dma_start(bq_sb[:], bq_d.rearrange("(t p) -> p t", p=128))
        nc.sync.dma_start(bk_sb[:], bk_d.rearrange("(t p) -> p t", p=128))
        nc.sync.dma_start(bv_sb[:], bv_d.partition_broadcast(SK))
        nc.sync.dma_start(bo_sb[:], bo_d.partition_broadcast(128))

        # Phase 0: kT[E, Sk] and V[Sk, E] (k/v biases folded via y ones-row).
        with tc.tile_pool(name="ph0", bufs=1) as ph0, tc.tile_pool(
            name="ph0ps", bufs=8, space="PSUM"
        ) as ph0ps:
            yT_sb = ph0.tile([128, CT, SKP], F32R)
            wk_sb = ph0.tile([128, CT, E], F32R)
            wv_sb = ph0.tile([128, CT, H * 64], F32R)
            yT_r = yT_d.rearrange("(t p) n -> p t n", p=128)
            wk_r = wk_d.rearrange("(t p) n -> p t n", p=128)
            wv_r = wv_d.rearrange("(t p) n -> p t n", p=128)
            wq_r = wq_d.rearrange("(t p) n -> p t n", p=128)
            wo_r = wo_d.rearrange("(t p) n -> p t n", p=128)
            # Per-k-tile weight loads so the t-major matmul loops below can
            # start as soon as the first tiles land.
            nc.sync.dma_start(yT_sb[:], yT_r)
            for t in range(CT):
                nc.sync.dma_start(wk_sb[:, t, :], wk_r[:, t, :])
                nc.sync.dma_start(wv_sb[:, t, :], wv_r[:, t, :])
            for lo, hi in ((0, 4), (4, 8)):
                nc.sync.dma_start(wq_sb[:, lo:hi, :], wq_r[:, lo:hi, :])
            psk = [
                ph0ps.tile([128, SKP], F32, tag="ph0", name=f"psk{i}")
                for i in range(ET)
            ]
            for t in range(CT):
                for et in range(ET):
                    nc.tensor.matmul(
                        psk[et][:],
                        wk_sb[:, t, et * 128 : (et + 1) * 128],
                        yT_sb[:, t, :],
                        start=(t == 0),
                        stop=(t == CT - 1),
                    )
            for et in range(ET):
                nc.scalar.activation(
                    kT_sb[:, et, :],
                    psk[et][:],
                    mybir.ActivationFunctionType.Identity,
                    bias=bk_sb[:, et : et + 1],
                )
            psv = [
                ph0ps.tile([SK, CHUNK], F32, tag="ph0", name=f"psv{i}")
                for i in range(2)
            ]
            for t in range(CT):
                for g in range(2):
                    nc.tensor.matmul(
                        psv[g][:],
                        yT_sb[:, t, 0:SK],
                        wv_sb[:, t, g * CHUNK : (g + 1) * CHUNK],
                        start=(t == 0),
                        stop=(t == CT - 1),
                    )
            for g in range(2):
                nc.vector.tensor_tensor(
                    v_sb[:, g * CHUNK : (g + 1) * CHUNK],
                    psv[g][:],
                    bv_sb[:, g * CHUNK : (g + 1) * CHUNK],
                    mybir.AluOpType.add,
                )

        # Main loop over row chunks. Emission order per chunk:
        #   QT(c) -> final(c-1) -> attention(c)
        # keeps the PE busy with QT matmuls while chunk c-1's normalization
        # (DVE/DMA) completes.
        xT_pool = ctx.enter_context(tc.tile_pool(name="xT", bufs=2))
        qT_pool = ctx.enter_context(tc.tile_pool(name="qT", bufs=2))
        oT_pool = ctx.enter_context(tc.tile_pool(name="oT", bufs=2))
        exps_pool = ctx.enter_context(tc.tile_pool(name="exps", bufs=4))
        tmpb_pool = ctx.enter_context(tc.tile_pool(name="tmpb", bufs=3))
        recip_pool = ctx.enter_context(tc.tile_pool(name="recip", bufs=2))
        outs_pool = ctx.enter_context(tc.tile_pool(name="outs", bufs=2))
        ps_q = ctx.enter_context(tc.tile_pool(name="ps_q", bufs=2, space="PSUM"))
        ps_s = ctx.enter_context(tc.tile_pool(name="ps_s", bufs=2, space="PSUM"))
        ps_den = ctx.enter_context(tc.tile_pool(name="ps_den", bufs=1, space="PSUM"))
        ps_av = ctx.enter_context(tc.tile_pool(name="ps_av", bufs=2, space="PSUM"))
        ps_f = ctx.enter_context(tc.tile_pool(name="ps_f", bufs=1, space="PSUM"))

        def emit_final_group(c, oT_sb, i):
            qt, n0 = i // 2, (i % 2) * CHUNK
            # Alternate between ps_f and the (idle during attention) ps_q
            # slots: double-buffers the final groups at zero PSUM-bank cost.
            if i % 2 == 0:
                ps = ps_f.tile([128, CHUNK], F32, tag="psf")
            else:
                ps = ps_q.tile([128, CHUNK], F32, tag="psq", name="psfq")
            for t in range(ET):
                nc.tensor.matmul(
                    ps[:],
                    oT_sb[:, t, qt * 128 : (qt + 1) * 128],
                    wo_sb[:, t, n0 : n0 + CHUNK],
                    start=(t == 0),
                    stop=(t == ET - 1),
                )
            o_sb = outs_pool.tile([128, CHUNK], F32, tag="osb")
            nc.vector.tensor_tensor(
                o_sb[:], ps[:], bo_sb[:, n0 : n0 + CHUNK], mybir.AluOpType.add
            )
            r0 = c * CHUNK + qt * 128
            nc.sync.dma_start(out_d[r0 : r0 + 128, n0 : n0 + CHUNK], o_sb[:])

        def load_xT(c):
            xT_sb = xT_pool.tile([128, ET, CHUNK], F32R, tag="xT")
            nc.sync.dma_start(xT_sb[:], xT_d[c])
            return xT_sb

        def emit_norm_recip(pden):
            # Fast approximate reciprocal (~18 bits; denominators are sums of
            # positive exps, bounded well away from 0/inf), then round to
            # fp32r for the select-matmul rhs. ~3x faster than the exact
            # reciprocal, which otherwise stalls the first rb matmul.
            rec32 = recip_pool.tile([H, CHUNK], F32, tag="recip32")
            nc.vector.reciprocal_approx_fast(rec32[:], pden[:])
            recip = recip_pool.tile([H, CHUNK], F32R, tag="recip")
            with nc.allow_low_precision(reason="fp32r feeds select-matmul"):
                nc.vector.tensor_copy(recip[:], rec32[:])
            return recip

        def emit_norm_pair(oT_sb, recip, et):
            # Broadcast recip rows (2*et, 2*et+1) across the pair's 128
            # partitions with a one-hot select matmul, then divide in place.
            rb = ps_s.tile([128, CHUNK], F32, tag="pss")
            nc.tensor.matmul(
                rb[:],
                sel_sb[:, et * 128 : (et + 1) * 128],
                recip[:],
                start=True,
                stop=True,
            )
            nc.vector.tensor_tensor(
                oT_sb[0:64, et, :],
                oT_sb[0:64, et, :],
                rb[0:64, :],
                mybir.AluOpType.mult,
            )
            nc.vector.tensor_tensor(
                oT_sb[64:128, et, :],
                oT_sb[64:128, et, :],
                rb[64:128, :],
                mybir.AluOpType.mult,
            )

        prev = None  # (c, oT_sb): chunk awaiting its final projection
        norm = None  # (pden, oT_sb): chunk awaiting softmax normalization
        xT_cur = load_xT(0)
        # wo is first needed by final(0) during attention(1); issuing its
        # load after xT(0) keeps it off QT(0)'s critical DMA path.
        wo_r2 = wo_d.rearrange("(t p) n -> p t n", p=128)
        for lo, hi in ((0, 4), (4, 8)):
            nc.sync.dma_start(wo_sb[:, lo:hi, :], wo_r2[:, lo:hi, :])
        for c in range(NCHUNK):
            xT_sb = xT_cur
            if c + 1 < NCHUNK:
                xT_cur = load_xT(c + 1)
            qT_sb = qT_pool.tile([128, ET, CHUNK], F32R, tag="qT")
            if norm is not None:
                n_recip = emit_norm_recip(norm[0])
            # Chunk c-1's softmax normalization is interleaved between the
            # (independent) QT groups so its PE<->DVE ping-pong never leaves
            # the tensor engine without queued work.
            for et in range(ET):
                ps = ps_q.tile([128, CHUNK], F32, tag="psq")
                for t in range(ET):
                    nc.tensor.matmul(
                        ps[:],
                        wq_sb[:, t, et * 128 : (et + 1) * 128],
                        xT_sb[:, t, :],
                        start=(t == 0),
                        stop=(t == ET - 1),
                    )
                nc.scalar.activation(
                    qT_sb[:, et, :],
                    ps[:],
                    mybir.ActivationFunctionType.Identity,
                    bias=bq_sb[:, et : et + 1],
                )
                if norm is not None and et >= 2:
                    emit_norm_pair(norm[1], n_recip, et - 2)
            if norm is not None:
                emit_norm_pair(norm[1], n_recip, 6)
                emit_norm_pair(norm[1], n_recip, 7)
            norm = None

            # Attention for chunk c, interleaved with chunk c-1's output
            # projection: the final-matmul groups have no dependency on this
            # chunk's exps, so they keep the PE busy (and the HAM clock-gate
            # warm) while the ScalarE exp of each head pair is in flight.
            #
            # fp32r matmuls must write PSUM at base partition 0, so each head
            # gets its own [64, CHUNK] attnV tile; the even head of a pair is
            # evicted to oT[0:64] by the DVE, the odd head by a
            # partition-shifting SBUF<-SBUF DMA to oT[64:128]. The softmax
            # division happens afterwards, in place on oT.
            pden = ps_den.tile([H, CHUNK], F32, tag="psden")
            oT_sb = oT_pool.tile([128, ET, CHUNK], F32R, tag="oT")
            for et in range(ET):
                hA, hB = 2 * et, 2 * et + 1
                psa = ps_s.tile([SK, CHUNK], F32, tag="pss")
                psb = ps_s.tile([SK, CHUNK], F32, tag="pss")
                # Adjacent score matmuls target PE row groups 0/64 and can
                # overlap in the array.
                nc.tensor.matmul(
                    psa[:], kT_sb[0:64, et, 0:SK], qT_sb[0:64, et, :],
                    start=True, stop=True,
                )
                nc.tensor.matmul(
                    psb[:], kT_sb[64:128, et, 0:SK], qT_sb[64:128, et, :],
                    start=True, stop=True,
                )
                exa = exps_pool.tile([SK, CHUNK], F32R, tag="exps")
                exb = exps_pool.tile([SK, CHUNK], F32R, tag="exps")
                nc.scalar.activation(exa[:], psa[:], mybir.ActivationFunctionType.Exp)
                nc.scalar.activation(exb[:], psb[:], mybir.ActivationFunctionType.Exp)
                for h, ex in ((hA, exa), (hB, exb)):
                    nc.tensor.matmul(
                        pden[:],
                        oh_sb[:, h * H : (h + 1) * H],
                        ex[:],
                        start=(h == 0),
                        stop=(h == H - 1),
                    )
                    pav = ps_av.tile([64, CHUNK], F32, tag="psav")
                    nc.tensor.matmul(
                        pav[:],
                        v_sb[:, h * 64 : (h + 1) * 64],
                        ex[:],
                        start=True,
                        stop=True,
                    )
                    if h == hA:
                        nc.scalar.activation(
                            oT_sb[0:64, et, :],
                            pav[:],
                            mybir.ActivationFunctionType.Identity,
                        )
                    else:
                        tmpb = tmpb_pool.tile([64, CHUNK], F32R, tag="tmpb")
                        nc.vector.tensor_copy(tmpb[:], pav[:])
                        nc.sync.dma_start(oT_sb[64:128, et, :], tmpb[:])
                if prev is not None and et >= 1:
                    emit_final_group(prev[0], prev[1], et - 1)
            if prev is not None:
                emit_final_group(prev[0], prev[1], 7)
            norm = (pden, oT_sb)
            prev = (c, oT_sb)
        # Tail: normalize and project the last chunk.
        n_recip = emit_norm_recip(norm[0])
        for et in range(ET):
            emit_norm_pair(norm[1], n_recip, et)
        for i in range(8):
            emit_final_group(prev[0], prev[1], i)

    nc.compile()
    return nc


def _get_program():
    global _PROGRAM
    if _PROGRAM is None:
        _PROGRAM = _build_program()
    return _PROGRAM


def kernel(x, y, wq, bq, wk, bk, wv, bv, wo, bo):
    x = np.asarray(x, dtype=np.float32)
    y = np.asarray(y, dtype=np.float32)
    wq = np.asarray(wq, dtype=np.float32)
    bq = np.asarray(bq, dtype=np.float32)
    wk = np.asarray(wk, dtype=np.float32)
    bk = np.asarray(bk, dtype=np.float32)
    wv = np.asarray(wv, dtype=np.float32)
    bv = np.asarray(bv, dtype=np.float32)
    wo = np.asarray(wo, dtype=np.float32)
    bo = np.asarray(bo, dtype=np.float32)

    scale = np.float32(1.0 / np.sqrt(D))
    wq_s = _round_f32r(wq * scale)
    bq_s = (bq * scale).astype(np.float32)

    wk_r = _round_f32r(wk)
    wv_r2 = _round_f32r(wv)

    wo_r = _round_f32r(wo)

    onehot = np.zeros((SK, H, H), dtype=np.float32)
    for h in range(H):
        onehot[:, h, h] = 1.0
    onehot = onehot.reshape(SK, H * H)

    sel = np.zeros((H, ET, 128), dtype=np.float32)
    for et in range(ET):
        sel[2 * et, et, 0:64] = 1.0
        sel[2 * et + 1, et, 64:128] = 1.0
    sel = sel.reshape(H, ET * 128)

    nc = _get_program()
    in_maps = []
    for b in range(N_CORES):
        # [E, Sq] -> [chunk, partition, e-tile, col], contiguous per chunk.
        xT = _round_f32r(
            np.ascontiguousarray(
                x[b].T.reshape(ET, 128, NCHUNK, CHUNK).transpose(2, 1, 0, 3)
            )
        )
        yT = np.zeros((C, SKP), dtype=np.float32)
        yT[:, :SK] = y[b].T
        yT = _round_f32r(yT)
        in_maps.append(
            {
                "xT": xT,
                "yT": yT,
                "wq": wq_s,
                "bq": bq_s,
                "wk": wk_r,
                "bk": bk.astype(np.float32),
                "wv": wv_r2,
                "bv": bv.astype(np.float32),
                "wo": wo_r,
                "bo": bo,
                "oh": onehot,
                "sel": sel,
            }
        )

    trace = bool(int(os.environ.get("KERNEL_TRACE", "0")))
    kwargs = {}
    if trace:
        kwargs = {"trace": True, "tmpdir": os.environ.get("KERNEL_TRACE_DIR")}
    try:
        res = run_bass_kernel_spmd(nc, in_maps, list(range(N_CORES)), **kwargs)
    except Exception:
        # The axon-tunneled devices occasionally report a transient
        # NRT_EXEC_UNIT_UNRECOVERABLE; a retry on the same executable has
        # been observed to succeed.
        res = run_bass_kernel_spmd(nc, in_maps, list(range(N_CORES)), **kwargs)
    if trace:
        kernel.last_exec_time_ns = res.exec_time_ns
        kernel.last_results = res
    out = np.stack([res.results[b]["out"] for b in range(N_CORES)])
    return np.ascontiguousarray(out)



# revision 13
# speedup vs baseline: 1.0425x; 1.0425x over previous
"""Cross-attention kernel for Trainium2 (8 NeuronCores, data-parallel over batch).

Problem (hardcoded): B=8, Sq=4096, Sk=77, E=1024, C=768, H=16 heads, D=64.

    q = x @ wq + bq; k = y @ wk + bk; v = y @ wv + bv
    out = softmax(q k^T / sqrt(D)) v @ wo + bo

Sharding: batch element b -> core b. No collectives.

All matmul operands are bf16 (PSUM accumulation stays fp32): same PE rate as
fp32r (1 col/cycle, ~216 ns per [128x128]x[128x512] MM) but half the DMA
bytes and FWL weight loads. End-to-end numerics sit at ~6e-3 rel-to-scale
(tolerance 2e-2), verified against a host-side rounding simulation.

Per-core pipeline (all matmuls contract over the SBUF partition dim):
  - qT[E, Sq] produced chunk-by-chunk from feature-major xT (host-prepared),
    evicted from PSUM by ScalarE with the bq bias (scale 1/sqrt(D) folded
    into wq/bq host-side).
  - Phase 0 builds kT[E, Sk] and V[Sk, *] on-chip from yT; k/v biases are
    folded by augmenting y with a ones-row host-side. V is laid out
    [Sk, H*128] = per head [V_h (64 cols) | ones (64 cols)] so that a single
    attnV matmul per head yields PSUM [att 0:64 | den 64:128]: the softmax
    denominator lands broadcast across partitions 64:128 for free.
  - scores^T[Sk, q] per head via lhsT=kT head slice (row groups 0/64);
    exp on ScalarE (no max-subtraction: scores are O(5), fp32 PSUM).
  - normalization: DVE fast-reciprocal of the den half (partitions 64:128),
    one SBUF->SBUF DMA per head-pair shifts the recip to partitions 0:64,
    then the PSUM eviction multiply (DVE) divides. Odd heads write oT[64:128]
    directly -- DVE ops may write a different 64-aligned partition base than
    they read as long as both INPUTS share a base (HW-verified).
  - final projection in transposed layout outT[E, Sq]: lhsT=wo tiles,
    rhs=oT tiles, bias bo added by ScalarE at eviction (per-partition).
    The host transposes outT back when assembling the full output.
  - emission interleaves, per 512-row chunk c and per et in 0..7:
    QT(c+1) group et (8 MMs) -> attention(c) pair et (4 MMs) ->
    final(c-1) group et (8 MMs), keeping the PE queue dense so ScalarE/DVE
    latency never drains it. ~20 dummy warmup MMs at t=0 bridge the initial
    weight DMA so the PE HAM clock-gate reaches 8/8 before real work.
"""

import os
from contextlib import ExitStack

import numpy as np
import ml_dtypes

import concourse.bass as bass
import concourse.tile as tile
from concourse import bacc, mybir
from concourse.bass_utils import run_bass_kernel_spmd

N_CORES = 8
SQ = 4096
SK = 77
SKP = 80  # padded free size for phase-0 kT psum tiles
E = 1024
C = 768
CA = C + 1  # y augmented with a ones-row (folds bk/bv)
H = 16
D = 64
CHUNK = 512
NCHUNK = SQ // CHUNK  # 8
ET = E // 128  # 8 e-tiles
F32 = mybir.dt.float32
BF16 = mybir.dt.bfloat16
BF = ml_dtypes.bfloat16

N_WARM = 20  # dummy PE warmup matmuls bridging the prologue DMA

_PROGRAM = None


def _build_program():
    nc = bacc.Bacc(
        "TRN2", target_bir_lowering=False, debug=False, num_devices=N_CORES
    )
    dz_d = nc.dram_tensor("dz", [128, 128 + CHUNK], BF16, kind="ExternalInput").ap()
    # xT pre-tiled host-side: [chunk, partition, e-tile, col]; 1MB per chunk.
    xT_d = nc.dram_tensor(
        "xT", [NCHUNK, 128, ET, CHUNK], BF16, kind="ExternalInput"
    ).ap()
    wq_d = nc.dram_tensor("wq", [E, E], BF16, kind="ExternalInput").ap()
    bq_d = nc.dram_tensor("bq", [E], F32, kind="ExternalInput").ap()
    yT_d = nc.dram_tensor("yT", [CA, SK], BF16, kind="ExternalInput").ap()
    wk_d = nc.dram_tensor("wk", [CA, E], BF16, kind="ExternalInput").ap()
    wv_d = nc.dram_tensor("wv", [CA, H * 64], BF16, kind="ExternalInput").ap()
    vones_d = nc.dram_tensor("vones", [SK, H, 64], BF16, kind="ExternalInput").ap()
    # wo pre-arranged for the transposed final: [p, t, eb, col]
    wo_d = nc.dram_tensor("wo", [128, ET, ET, 128], BF16, kind="ExternalInput").ap()
    bo_d = nc.dram_tensor("bo", [E], F32, kind="ExternalInput").ap()
    outT_d = nc.dram_tensor("outT", [ET, 128, SQ], F32, kind="ExternalOutput").ap()

    with tile.TileContext(nc) as tc, ExitStack() as ctx:
        consts = ctx.enter_context(tc.tile_pool(name="consts", bufs=1))
        dz_sb = consts.tile([128, 128 + CHUNK], BF16)
        wq_sb = consts.tile([128, ET, E], BF16)
        wo_sb = consts.tile([128, ET, ET, 128], BF16)
        kT_sb = consts.tile([128, ET, SKP], BF16)
        v_sb = consts.tile([SK, H, 128], BF16)
        bq_sb = consts.tile([128, ET], F32)
        bo_sb = consts.tile([128, ET], F32)

        # Prologue DMA order = consumption order: warmup tile, xT(0), wq
        # (per-et column blocks so QT(0) group 0 starts after ~1.25MB), then
        # the phase-0 inputs, then wo (first needed by final(0) in chunk 1).
        nc.sync.dma_start(dz_sb[:], dz_d)

        xT_pool = ctx.enter_context(tc.tile_pool(name="xT", bufs=2))
        qT_pool = ctx.enter_context(tc.tile_pool(name="qT", bufs=2))
        oT_pool = ctx.enter_context(tc.tile_pool(name="oT", bufs=2))
        exps_pool = ctx.enter_context(tc.tile_pool(name="exps", bufs=4))
        rbhi_pool = ctx.enter_context(tc.tile_pool(name="rbhi", bufs=3))
        rblo_pool = ctx.enter_context(tc.tile_pool(name="rblo", bufs=3))
        outs_pool = ctx.enter_context(tc.tile_pool(name="outs", bufs=3))
        ps_qf = ctx.enter_context(tc.tile_pool(name="ps_qf", bufs=3, space="PSUM"))
        ps_s = ctx.enter_context(tc.tile_pool(name="ps_s", bufs=2, space="PSUM"))
        ps_av = ctx.enter_context(tc.tile_pool(name="ps_av", bufs=3, space="PSUM"))

        def load_xT(c):
            xT_sb = xT_pool.tile([128, ET, CHUNK], BF16, tag="xT")
            nc.sync.dma_start(xT_sb[:], xT_d[c])
            return xT_sb

        xT_cur = load_xT(0)
        wq_r = wq_d.rearrange("(t p) n -> p t n", p=128)
        for et in range(ET):
            nc.sync.dma_start(
                wq_sb[:, :, et * 128 : (et + 1) * 128],
                wq_r[:, :, et * 128 : (et + 1) * 128],
            )
        nc.sync.dma_start(bq_sb[:], bq_d.rearrange("(t p) -> p t", p=128))

        # PE warmup: garbage-free dummy accumulations on the zero tile.
        ps_warm = ps_av.tile([128, CHUNK], F32, tag="psav", name="warm")
        for i in range(N_WARM):
            nc.tensor.matmul(
                ps_warm[:],
                dz_sb[:, 0:128],
                dz_sb[:, 128 : 128 + CHUNK],
                start=(i == 0),
                stop=(i == N_WARM - 1),
            )

        # Phase-0 input DMAs (consumed after QT(0) below).
        ph0 = ctx.enter_context(tc.tile_pool(name="ph0", bufs=1))
        CTT = (CA + 127) // 128  # 7 contraction tiles; last has 65 rows
        yT_sb = ph0.tile([128, CTT, SK], BF16)
        wk_sb = ph0.tile([128, CTT, E], BF16)
        wv_sb = ph0.tile([128, CTT, H * 64], BF16)
        for t in range(CTT):
            rows = min(128, CA - t * 128)
            nc.sync.dma_start(yT_sb[0:rows, t, :], yT_d[t * 128 : t * 128 + rows, :])
            nc.sync.dma_start(wk_sb[0:rows, t, :], wk_d[t * 128 : t * 128 + rows, :])
            nc.sync.dma_start(wv_sb[0:rows, t, :], wv_d[t * 128 : t * 128 + rows, :])
        nc.sync.dma_start(v_sb[:, :, 0:64], vones_d)
        for t in range(ET):
            nc.sync.dma_start(wo_sb[:, t, :, :], wo_d[:, t, :, :])
        nc.sync.dma_start(bo_sb[:], bo_d.rearrange("(t p) -> p t", p=128))

        def emit_qt_group(xT_sb, qT_sb, et):
            ps = ps_qf.tile([128, CHUNK], F32, tag="psqf")
            for t in range(ET):
                nc.tensor.matmul(
                    ps[:],
                    wq_sb[:, t, et * 128 : (et + 1) * 128],
                    xT_sb[:, t, :],
                    start=(t == 0),
                    stop=(t == ET - 1),
                )
            nc.scalar.activation(
                qT_sb[:, et, :],
                ps[:],
                mybir.ActivationFunctionType.Identity,
                bias=bq_sb[:, et : et + 1],
            )

        # QT(0): 8 groups, standalone (phase 0 weights still streaming in).
        qT_cur = qT_pool.tile([128, ET, CHUNK], BF16, tag="qT")
        for et in range(ET):
            emit_qt_group(xT_cur, qT_cur, et)

        # Phase 0: kT[E, Sk] and V[Sk, H*128] (biases via y ones-row).
        for et in range(ET):
            psk = ps_s.tile([128, SKP], F32, tag="pss", name="psk")
            for t in range(CTT):
                rows = min(128, CA - t * 128)
                nc.tensor.matmul(
                    psk[:, 0:SK],
                    wk_sb[0:rows, t, et * 128 : (et + 1) * 128],
                    yT_sb[0:rows, t, :],
                    start=(t == 0),
                    stop=(t == CTT - 1),
                )
            with nc.allow_low_precision(reason="bf16 kT operand"):
                nc.vector.tensor_copy(kT_sb[:, et, 0:SK], psk[:, 0:SK])
        for g in range(2):
            psv = ps_av.tile([SK, CHUNK], F32, tag="psav", name="psv")
            for t in range(CTT):
                rows = min(128, CA - t * 128)
                nc.tensor.matmul(
                    psv[:],
                    yT_sb[0:rows, t, :],
                    wv_sb[0:rows, t, g * CHUNK : (g + 1) * CHUNK],
                    start=(t == 0),
                    stop=(t == CTT - 1),
                )
            for hh in range(8):
                h = g * 8 + hh
                with nc.allow_low_precision(reason="bf16 V operand"):
                    nc.vector.tensor_copy(
                        v_sb[:, h, 64:128], psv[:, hh * 64 : (hh + 1) * 64]
                    )

        def emit_attn_pair(qT_sb, oT_sb, et):
            hA, hB = 2 * et, 2 * et + 1
            psa = ps_s.tile([SK, CHUNK], F32, tag="pss")
            psb = ps_s.tile([SK, CHUNK], F32, tag="pss")
            nc.tensor.matmul(
                psa[:], kT_sb[0:64, et, 0:SK], qT_sb[0:64, et, :],
                start=True, stop=True,
            )
            nc.tensor.matmul(
                psb[:], kT_sb[64:128, et, 0:SK], qT_sb[64:128, et, :],
                start=True, stop=True,
            )
            exa = exps_pool.tile([SK, CHUNK], BF16, tag="exps")
            exb = exps_pool.tile([SK, CHUNK], BF16, tag="exps")
            nc.scalar.activation(exa[:], psa[:], mybir.ActivationFunctionType.Exp)
            nc.scalar.activation(exb[:], psb[:], mybir.ActivationFunctionType.Exp)
            pava = ps_av.tile([128, CHUNK], F32, tag="psav")
            pavb = ps_av.tile([128, CHUNK], F32, tag="psav")
            nc.tensor.matmul(
                pava[:], v_sb[:, hA, :], exa[:], start=True, stop=True
            )
            nc.tensor.matmul(
                pavb[:], v_sb[:, hB, :], exb[:], start=True, stop=True
            )
            # PSUM layout per head is [den 0:64 | att 64:128] (lhsT=[ones|V]).
            # fast-reciprocal only works at base partition 0, so recip the den
            # halves there, DMA-shift the pair up to partitions 64:128, and
            # evict with a multiply whose inputs are both base-64 (the dst
            # may sit at either 64-aligned base; HW-verified).
            rblo = rblo_pool.tile([128, 2 * CHUNK], F32, tag="rblo")
            rbhi = rbhi_pool.tile([128, 2 * CHUNK], F32, tag="rbhi")
            nc.vector.reciprocal_approx_fast(rblo[0:64, 0:CHUNK], pava[0:64, :])
            nc.vector.reciprocal_approx_fast(
                rblo[0:64, CHUNK : 2 * CHUNK], pavb[0:64, :]
            )
            nc.sync.dma_start(rbhi[64:128, :], rblo[0:64, :])
            with nc.allow_low_precision(reason="bf16 attention output"):
                nc.vector.tensor_tensor(
                    oT_sb[0:64, et, :], pava[64:128, :], rbhi[64:128, 0:CHUNK],
                    mybir.AluOpType.mult,
                )
                nc.vector.tensor_tensor(
                    oT_sb[64:128, et, :], pavb[64:128, :],
                    rbhi[64:128, CHUNK : 2 * CHUNK], mybir.AluOpType.mult,
                )

        def emit_final_group(c, oT_sb, eb):
            ps = ps_qf.tile([128, CHUNK], F32, tag="psqf", name="psf")
            for t in range(ET):
                nc.tensor.matmul(
                    ps[:],
                    wo_sb[:, t, eb, :],
                    oT_sb[:, t, :],
                    start=(t == 0),
                    stop=(t == ET - 1),
                )
            o_sb = outs_pool.tile([128, CHUNK], F32, tag="osb")
            nc.scalar.activation(
                o_sb[:],
                ps[:],
                mybir.ActivationFunctionType.Identity,
                bias=bo_sb[:, eb : eb + 1],
            )
            nc.sync.dma_start(
                outT_d[eb, :, c * CHUNK : (c + 1) * CHUNK], o_sb[:]
            )

        # Main software pipeline over chunks:
        #   body(c): per et: QT(c+1) group et | attn(c) pair et |
        #            final(c-1) group et
        prev = None  # (c, oT_sb) awaiting final projection
        for c in range(NCHUNK):
            xT_sb = xT_cur
            qT_sb = qT_cur
            if c + 1 < NCHUNK:
                xT_cur = load_xT(c + 1)
                qT_cur = qT_pool.tile([128, ET, CHUNK], BF16, tag="qT")
            oT_sb = oT_pool.tile([128, ET, CHUNK], BF16, tag="oT")
            for et in range(ET):
                if c + 1 < NCHUNK:
                    emit_qt_group(xT_cur, qT_cur, et)
                emit_attn_pair(qT_sb, oT_sb, et)
                if prev is not None:
                    emit_final_group(prev[0], prev[1], et)
            prev = (c, oT_sb)
        for eb in range(ET):
            emit_final_group(prev[0], prev[1], eb)

    nc.compile()
    return nc


def _get_program():
    global _PROGRAM
    if _PROGRAM is None:
        _PROGRAM = _build_program()
    return _PROGRAM


def kernel(x, y, wq, bq, wk, bk, wv, bv, wo, bo):
    x = np.asarray(x, dtype=np.float32)
    y = np.asarray(y, dtype=np.float32)
    wq = np.asarray(wq, dtype=np.float32)
    bq = np.asarray(bq, dtype=np.float32)
    wk = np.asarray(wk, dtype=np.float32)
    bk = np.asarray(bk, dtype=np.float32)
    wv = np.asarray(wv, dtype=np.float32)
    bv = np.asarray(bv, dtype=np.float32)
    wo = np.asarray(wo, dtype=np.float32)
    bo = np.asarray(bo, dtype=np.float32)

    scale = np.float32(1.0 / np.sqrt(D))
    wq_s = (wq * scale).astype(BF)
    bq_s = (bq * scale).astype(np.float32)

    # Fold k/v biases via an appended ones-row on y.
    wk_a = np.concatenate([wk, bk[None, :]], axis=0).astype(BF)
    wv_a = np.concatenate([wv, bv[None, :]], axis=0).astype(BF)
    wo_r = np.ascontiguousarray(
        wo.reshape(ET, 128, ET, 128).transpose(1, 0, 2, 3)
    ).astype(BF)
    vones = np.ones((SK, H, 64), dtype=BF)
    dz = np.zeros((128, 128 + CHUNK), dtype=BF)

    nc = _get_program()
    in_maps = []
    for b in range(N_CORES):
        xT = np.ascontiguousarray(
            x[b].T.reshape(ET, 128, NCHUNK, CHUNK).transpose(2, 1, 0, 3)
        ).astype(BF)
        yT = np.concatenate(
            [y[b].T, np.ones((1, SK), dtype=np.float32)], axis=0
        ).astype(BF)
        in_maps.append(
            {
                "dz": dz,
                "xT": xT,
                "wq": wq_s,
                "bq": bq_s,
                "yT": yT,
                "wk": wk_a,
                "wv": wv_a,
                "vones": vones,
                "wo": wo_r,
                "bo": bo,
            }
        )

    trace = bool(int(os.environ.get("KERNEL_TRACE", "0")))
    kwargs = {}
    if trace:
        kwargs = {"trace": True, "tmpdir": os.environ.get("KERNEL_TRACE_DIR")}
    try:
        res = run_bass_kernel_spmd(nc, in_maps, list(range(N_CORES)), **kwargs)
    except Exception:
        # The axon-tunneled devices occasionally report a transient
        # NRT_EXEC_UNIT_UNRECOVERABLE; a retry on the same executable has
        # been observed to succeed.
        res = run_bass_kernel_spmd(nc, in_maps, list(range(N_CORES)), **kwargs)
    if trace:
        kernel.last_exec_time_ns = res.exec_time_ns
        kernel.last_results = res
    out = np.stack(
        [
            np.ascontiguousarray(
                res.results[b]["outT"].reshape(E, SQ).T
            )
            for b in range(N_CORES)
        ]
    )
    return out


# revision 14
# speedup vs baseline: 1.4015x; 1.3444x over previous
"""Cross-attention kernel for Trainium2 (8 NeuronCores, data-parallel over batch).

Problem (hardcoded): B=8, Sq=4096, Sk=77, E=1024, C=768, H=16 heads, D=64.

    q = x @ wq + bq; k = y @ wk + bk; v = y @ wv + bv
    out = softmax(q k^T / sqrt(D)) v @ wo + bo

Sharding: batch element b -> core b. No collectives.

All matmul operands are bf16 (PSUM accumulation stays fp32): same PE rate as
fp32r (1 col/cycle, ~216 ns per [128x128]x[128x512] MM) but half the DMA
bytes and FWL weight loads. End-to-end numerics sit at ~6e-3 rel-to-scale
(tolerance 2e-2), verified against a host-side rounding simulation.

Per-core pipeline (all matmuls contract over the SBUF partition dim):
  - qT[E, Sq] produced chunk-by-chunk from feature-major xT (host-prepared),
    evicted from PSUM by ScalarE with the bq bias (scale 1/sqrt(D) folded
    into wq/bq host-side).
  - Phase 0 builds kT[E, Sk] and V[Sk, *] on-chip from yT; k/v biases are
    folded by augmenting y with a ones-row host-side. V is laid out
    [Sk, H*128] = per head [V_h (64 cols) | ones (64 cols)] so that a single
    attnV matmul per head yields PSUM [att 0:64 | den 64:128]: the softmax
    denominator lands broadcast across partitions 64:128 for free.
  - scores^T[Sk, q] per head via lhsT=kT head slice (row groups 0/64);
    exp on ScalarE (no max-subtraction: scores are O(5), fp32 PSUM).
  - normalization: DVE fast-reciprocal of the den half (partitions 64:128),
    one SBUF->SBUF DMA per head-pair shifts the recip to partitions 0:64,
    then the PSUM eviction multiply (DVE) divides. Odd heads write oT[64:128]
    directly -- DVE ops may write a different 64-aligned partition base than
    they read as long as both INPUTS share a base (HW-verified).
  - final projection in transposed layout outT[E, Sq]: lhsT=wo tiles,
    rhs=oT tiles, bias bo added by ScalarE at eviction (per-partition).
    The host transposes outT back when assembling the full output.
  - emission interleaves, per 512-row chunk c and per et in 0..7:
    QT(c+1) group et (8 MMs) -> attention(c) pair et (4 MMs) ->
    final(c-1) group et (8 MMs), keeping the PE queue dense so ScalarE/DVE
    latency never drains it. ~20 dummy warmup MMs at t=0 bridge the initial
    weight DMA so the PE HAM clock-gate reaches 8/8 before real work.
"""

import os
from contextlib import ExitStack

import numpy as np
import ml_dtypes

import concourse.bass as bass
import concourse.tile as tile
from concourse import bacc, mybir
from concourse.bass_utils import run_bass_kernel_spmd

N_CORES = 8
SQ = 4096
SK = 77
SKP = 80  # padded free size for phase-0 kT psum tiles
E = 1024
C = 768
CA = C + 1  # y augmented with a ones-row (folds bk/bv)
H = 16
D = 64
CHUNK = 512
NCHUNK = SQ // CHUNK  # 8
ET = E // 128  # 8 e-tiles
F32 = mybir.dt.float32
BF16 = mybir.dt.bfloat16
BF = ml_dtypes.bfloat16

N_WARM = 20  # dummy PE warmup matmuls bridging the prologue DMA

_PROGRAM = None


def _build_program():
    nc = bacc.Bacc(
        "TRN2", target_bir_lowering=False, debug=False, num_devices=N_CORES
    )
    dz_d = nc.dram_tensor("dz", [128, 128 + CHUNK], BF16, kind="ExternalInput").ap()
    # xT pre-tiled host-side: [chunk, partition, e-tile, col]; 1MB per chunk.
    xT_d = nc.dram_tensor(
        "xT", [NCHUNK, 128, ET, CHUNK], BF16, kind="ExternalInput"
    ).ap()
    wq_d = nc.dram_tensor("wq", [E, E], BF16, kind="ExternalInput").ap()
    bq_d = nc.dram_tensor("bq", [E], F32, kind="ExternalInput").ap()
    yT_d = nc.dram_tensor("yT", [CA, SK], BF16, kind="ExternalInput").ap()
    wk_d = nc.dram_tensor("wk", [CA, E], BF16, kind="ExternalInput").ap()
    wv_d = nc.dram_tensor("wv", [CA, H * 64], BF16, kind="ExternalInput").ap()
    vones_d = nc.dram_tensor("vones", [SK, H, 64], BF16, kind="ExternalInput").ap()
    # wo pre-arranged for the transposed final: [p, t, eb, col]
    wo_d = nc.dram_tensor("wo", [128, ET, ET, 128], BF16, kind="ExternalInput").ap()
    bo_d = nc.dram_tensor("bo", [E], F32, kind="ExternalInput").ap()
    outT_d = nc.dram_tensor("outT", [ET, 128, SQ], F32, kind="ExternalOutput").ap()

    with tile.TileContext(nc) as tc, ExitStack() as ctx:
        consts = ctx.enter_context(tc.tile_pool(name="consts", bufs=1))
        dz_sb = consts.tile([128, 128 + CHUNK], BF16)
        wq_sb = consts.tile([128, ET, E], BF16)
        wo_sb = consts.tile([128, ET, ET, 128], BF16)
        kT_sb = consts.tile([128, ET, SKP], BF16)
        v_sb = consts.tile([SK, H, 128], BF16)
        bq_sb = consts.tile([128, ET], F32)
        bo_sb = consts.tile([128, ET], F32)

        # Prologue DMA order = consumption order: warmup tile, xT(0), wq
        # (per-et column blocks so QT(0) group 0 starts after ~1.25MB), then
        # the phase-0 inputs, then wo (first needed by final(0) in chunk 1).
        nc.sync.dma_start(dz_sb[:], dz_d)

        xT_pool = ctx.enter_context(tc.tile_pool(name="xT", bufs=2))
        qT_pool = ctx.enter_context(tc.tile_pool(name="qT", bufs=2))
        oT_pool = ctx.enter_context(tc.tile_pool(name="oT", bufs=2))
        exps_pool = ctx.enter_context(tc.tile_pool(name="exps", bufs=4))
        rbhi_pool = ctx.enter_context(tc.tile_pool(name="rbhi", bufs=3))
        rblo_pool = ctx.enter_context(tc.tile_pool(name="rblo", bufs=3))
        outs_pool = ctx.enter_context(tc.tile_pool(name="outs", bufs=3))
        ps_qf = ctx.enter_context(tc.tile_pool(name="ps_qf", bufs=3, space="PSUM"))
        ps_s = ctx.enter_context(tc.tile_pool(name="ps_s", bufs=2, space="PSUM"))
        ps_av = ctx.enter_context(tc.tile_pool(name="ps_av", bufs=3, space="PSUM"))

        def load_xT(c):
            xT_sb = xT_pool.tile([128, ET, CHUNK], BF16, tag="xT")
            nc.sync.dma_start(xT_sb[:], xT_d[c])
            return xT_sb

        xT_cur = load_xT(0)
        wq_r = wq_d.rearrange("(t p) n -> p t n", p=128)
        for et in range(ET):
            nc.sync.dma_start(
                wq_sb[:, :, et * 128 : (et + 1) * 128],
                wq_r[:, :, et * 128 : (et + 1) * 128],
            )
        nc.sync.dma_start(bq_sb[:], bq_d.rearrange("(t p) -> p t", p=128))

        # PE warmup: garbage-free dummy accumulations on the zero tile.
        ps_warm = ps_av.tile([128, CHUNK], F32, tag="psav", name="warm")
        for i in range(N_WARM):
            nc.tensor.matmul(
                ps_warm[:],
                dz_sb[:, 0:128],
                dz_sb[:, 128 : 128 + CHUNK],
                start=(i == 0),
                stop=(i == N_WARM - 1),
            )

        # Phase-0 input DMAs (consumed after QT(0) below).
        ph0 = ctx.enter_context(tc.tile_pool(name="ph0", bufs=1))
        CTT = (CA + 127) // 128  # 7 contraction tiles; last has 65 rows
        yT_sb = ph0.tile([128, CTT, SK], BF16)
        wk_sb = ph0.tile([128, CTT, E], BF16)
        wv_sb = ph0.tile([128, CTT, H * 64], BF16)
        for t in range(CTT):
            rows = min(128, CA - t * 128)
            nc.sync.dma_start(yT_sb[0:rows, t, :], yT_d[t * 128 : t * 128 + rows, :])
            nc.sync.dma_start(wk_sb[0:rows, t, :], wk_d[t * 128 : t * 128 + rows, :])
            nc.sync.dma_start(wv_sb[0:rows, t, :], wv_d[t * 128 : t * 128 + rows, :])
        nc.sync.dma_start(v_sb[:, :, 0:64], vones_d)
        for t in range(ET):
            nc.sync.dma_start(wo_sb[:, t, :, :], wo_d[:, t, :, :])
        nc.sync.dma_start(bo_sb[:], bo_d.rearrange("(t p) -> p t", p=128))

        def emit_qt_group(xT_sb, qT_sb, et):
            ps = ps_qf.tile([128, CHUNK], F32, tag="psqf")
            for t in range(ET):
                nc.tensor.matmul(
                    ps[:],
                    wq_sb[:, t, et * 128 : (et + 1) * 128],
                    xT_sb[:, t, :],
                    start=(t == 0),
                    stop=(t == ET - 1),
                )
            nc.scalar.activation(
                qT_sb[:, et, :],
                ps[:],
                mybir.ActivationFunctionType.Identity,
                bias=bq_sb[:, et : et + 1],
            )

        # QT(0): 8 groups, standalone (phase 0 weights still streaming in).
        qT_cur = qT_pool.tile([128, ET, CHUNK], BF16, tag="qT")
        for et in range(ET):
            emit_qt_group(xT_cur, qT_cur, et)

        # Phase 0: kT[E, Sk] and V[Sk, H*128] (biases via y ones-row).
        for et in range(ET):
            psk = ps_s.tile([128, SKP], F32, tag="pss", name="psk")
            for t in range(CTT):
                rows = min(128, CA - t * 128)
                nc.tensor.matmul(
                    psk[:, 0:SK],
                    wk_sb[0:rows, t, et * 128 : (et + 1) * 128],
                    yT_sb[0:rows, t, :],
                    start=(t == 0),
                    stop=(t == CTT - 1),
                )
            with nc.allow_low_precision(reason="bf16 kT operand"):
                nc.vector.tensor_copy(kT_sb[:, et, 0:SK], psk[:, 0:SK])
        for g in range(2):
            psv = ps_av.tile([SK, CHUNK], F32, tag="psav", name="psv")
            for t in range(CTT):
                rows = min(128, CA - t * 128)
                nc.tensor.matmul(
                    psv[:],
                    yT_sb[0:rows, t, :],
                    wv_sb[0:rows, t, g * CHUNK : (g + 1) * CHUNK],
                    start=(t == 0),
                    stop=(t == CTT - 1),
                )
            for hh in range(8):
                h = g * 8 + hh
                with nc.allow_low_precision(reason="bf16 V operand"):
                    nc.vector.tensor_copy(
                        v_sb[:, h, 64:128], psv[:, hh * 64 : (hh + 1) * 64]
                    )

        def emit_attn_pair(qT_sb, oT_sb, et):
            hA, hB = 2 * et, 2 * et + 1
            psa = ps_s.tile([SK, CHUNK], F32, tag="pss")
            psb = ps_s.tile([SK, CHUNK], F32, tag="pss")
            nc.tensor.matmul(
                psa[:], kT_sb[0:64, et, 0:SK], qT_sb[0:64, et, :],
                start=True, stop=True,
            )
            nc.tensor.matmul(
                psb[:], kT_sb[64:128, et, 0:SK], qT_sb[64:128, et, :],
                start=True, stop=True,
            )
            exa = exps_pool.tile([SK, CHUNK], BF16, tag="exps")
            exb = exps_pool.tile([SK, CHUNK], BF16, tag="exps")
            nc.scalar.activation(exa[:], psa[:], mybir.ActivationFunctionType.Exp)
            nc.scalar.activation(exb[:], psb[:], mybir.ActivationFunctionType.Exp)
            pava = ps_av.tile([128, CHUNK], F32, tag="psav")
            pavb = ps_av.tile([128, CHUNK], F32, tag="psav")
            nc.tensor.matmul(
                pava[:], v_sb[:, hA, :], exa[:], start=True, stop=True
            )
            nc.tensor.matmul(
                pavb[:], v_sb[:, hB, :], exb[:], start=True, stop=True
            )
            # PSUM layout per head is [den 0:64 | att 64:128] (lhsT=[ones|V]).
            # fast-reciprocal only works at base partition 0, so recip the den
            # halves there, DMA-shift the pair up to partitions 64:128, and
            # evict with a multiply whose inputs are both base-64 (the dst
            # may sit at either 64-aligned base; HW-verified).
            rblo = rblo_pool.tile([128, 2 * CHUNK], F32, tag="rblo")
            rbhi = rbhi_pool.tile([128, 2 * CHUNK], F32, tag="rbhi")
            nc.vector.reciprocal_approx_fast(rblo[0:64, 0:CHUNK], pava[0:64, :])
            nc.vector.reciprocal_approx_fast(
                rblo[0:64, CHUNK : 2 * CHUNK], pavb[0:64, :]
            )
            # ScalarE moves the recip pair to partitions 64:128 (cross-base
            # writes are legal for single-input ops); ~4x cheaper end-to-end
            # than an SBUF->SBUF DMA whose completion receipt costs ~2us.
            nc.scalar.activation(
                rbhi[64:128, :], rblo[0:64, :],
                mybir.ActivationFunctionType.Identity,
            )
            with nc.allow_low_precision(reason="bf16 attention output"):
                nc.vector.tensor_tensor(
                    oT_sb[0:64, et, :], pava[64:128, :], rbhi[64:128, 0:CHUNK],
                    mybir.AluOpType.mult,
                )
                nc.vector.tensor_tensor(
                    oT_sb[64:128, et, :], pavb[64:128, :],
                    rbhi[64:128, CHUNK : 2 * CHUNK], mybir.AluOpType.mult,
                )

        def emit_final_group(c, oT_sb, eb):
            ps = ps_qf.tile([128, CHUNK], F32, tag="psqf", name="psf")
            for t in range(ET):
                nc.tensor.matmul(
                    ps[:],
                    wo_sb[:, t, eb, :],
                    oT_sb[:, t, :],
                    start=(t == 0),
                    stop=(t == ET - 1),
                )
            o_sb = outs_pool.tile([128, CHUNK], F32, tag="osb")
            nc.scalar.activation(
                o_sb[:],
                ps[:],
                mybir.ActivationFunctionType.Identity,
                bias=bo_sb[:, eb : eb + 1],
            )
            nc.sync.dma_start(
                outT_d[eb, :, c * CHUNK : (c + 1) * CHUNK], o_sb[:]
            )

        # Main software pipeline over chunks:
        #   body(c): per et: QT(c+1) group et | attn(c) pair et |
        #            final(c-1) group et
        prev = None  # (c, oT_sb) awaiting final projection
        for c in range(NCHUNK):
            xT_sb = xT_cur
            qT_sb = qT_cur
            if c + 1 < NCHUNK:
                xT_cur = load_xT(c + 1)
                qT_cur = qT_pool.tile([128, ET, CHUNK], BF16, tag="qT")
            oT_sb = oT_pool.tile([128, ET, CHUNK], BF16, tag="oT")
            for et in range(ET):
                if c + 1 < NCHUNK:
                    emit_qt_group(xT_cur, qT_cur, et)
                emit_attn_pair(qT_sb, oT_sb, et)
                if prev is not None:
                    emit_final_group(prev[0], prev[1], et)
            prev = (c, oT_sb)
        for eb in range(ET):
            emit_final_group(prev[0], prev[1], eb)

    nc.compile()
    return nc


def _get_program():
    global _PROGRAM
    if _PROGRAM is None:
        _PROGRAM = _build_program()
    return _PROGRAM


def kernel(x, y, wq, bq, wk, bk, wv, bv, wo, bo):
    x = np.asarray(x, dtype=np.float32)
    y = np.asarray(y, dtype=np.float32)
    wq = np.asarray(wq, dtype=np.float32)
    bq = np.asarray(bq, dtype=np.float32)
    wk = np.asarray(wk, dtype=np.float32)
    bk = np.asarray(bk, dtype=np.float32)
    wv = np.asarray(wv, dtype=np.float32)
    bv = np.asarray(bv, dtype=np.float32)
    wo = np.asarray(wo, dtype=np.float32)
    bo = np.asarray(bo, dtype=np.float32)

    scale = np.float32(1.0 / np.sqrt(D))
    wq_s = (wq * scale).astype(BF)
    bq_s = (bq * scale).astype(np.float32)

    # Fold k/v biases via an appended ones-row on y.
    wk_a = np.concatenate([wk, bk[None, :]], axis=0).astype(BF)
    wv_a = np.concatenate([wv, bv[None, :]], axis=0).astype(BF)
    wo_r = np.ascontiguousarray(
        wo.reshape(ET, 128, ET, 128).transpose(1, 0, 2, 3)
    ).astype(BF)
    vones = np.ones((SK, H, 64), dtype=BF)
    dz = np.zeros((128, 128 + CHUNK), dtype=BF)

    nc = _get_program()
    in_maps = []
    for b in range(N_CORES):
        xT = np.ascontiguousarray(
            x[b].T.reshape(ET, 128, NCHUNK, CHUNK).transpose(2, 1, 0, 3)
        ).astype(BF)
        yT = np.concatenate(
            [y[b].T, np.ones((1, SK), dtype=np.float32)], axis=0
        ).astype(BF)
        in_maps.append(
            {
                "dz": dz,
                "xT": xT,
                "wq": wq_s,
                "bq": bq_s,
                "yT": yT,
                "wk": wk_a,
                "wv": wv_a,
                "vones": vones,
                "wo": wo_r,
                "bo": bo,
            }
        )

    trace = bool(int(os.environ.get("KERNEL_TRACE", "0")))
    kwargs = {}
    if trace:
        kwargs = {"trace": True, "tmpdir": os.environ.get("KERNEL_TRACE_DIR")}
    try:
        res = run_bass_kernel_spmd(nc, in_maps, list(range(N_CORES)), **kwargs)
    except Exception:
        # The axon-tunneled devices occasionally report a transient
        # NRT_EXEC_UNIT_UNRECOVERABLE; a retry on the same executable has
        # been observed to succeed.
        res = run_bass_kernel_spmd(nc, in_maps, list(range(N_CORES)), **kwargs)
    if trace:
        kernel.last_exec_time_ns = res.exec_time_ns
        kernel.last_results = res
    out = np.stack(
        [
            np.ascontiguousarray(
                res.results[b]["outT"].reshape(E, SQ).T
            )
            for b in range(N_CORES)
        ]
    )
    return out


# revision 16
# speedup vs baseline: 1.4051x; 1.0026x over previous
"""Cross-attention kernel for Trainium2 (8 NeuronCores, data-parallel over batch).

Problem (hardcoded): B=8, Sq=4096, Sk=77, E=1024, C=768, H=16 heads, D=64.

    q = x @ wq + bq; k = y @ wk + bk; v = y @ wv + bv
    out = softmax(q k^T / sqrt(D)) v @ wo + bo

Sharding: batch element b -> core b. No collectives.

All matmul operands are bf16 (PSUM accumulation stays fp32): same PE rate as
fp32r (1 col/cycle, ~216 ns per [128x128]x[128x512] MM) but half the DMA
bytes and FWL weight loads. End-to-end numerics sit at ~6e-3 rel-to-scale
(tolerance 2e-2), verified against a host-side rounding simulation.

Per-core pipeline (all matmuls contract over the SBUF partition dim):
  - qT[E, Sq] produced chunk-by-chunk from feature-major xT (host-prepared),
    evicted from PSUM by ScalarE with the bq bias (scale 1/sqrt(D) folded
    into wq/bq host-side).
  - Phase 0 builds kT[E, Sk] and V[Sk, *] on-chip from yT; k/v biases are
    folded by augmenting y with a ones-row host-side. V is laid out
    [Sk, H*128] = per head [V_h (64 cols) | ones (64 cols)] so that a single
    attnV matmul per head yields PSUM [att 0:64 | den 64:128]: the softmax
    denominator lands broadcast across partitions 64:128 for free.
  - scores^T[Sk, q] per head via lhsT=kT head slice (row groups 0/64);
    exp on ScalarE (no max-subtraction: scores are O(5), fp32 PSUM).
  - normalization: DVE fast-reciprocal of the den half (partitions 64:128),
    one SBUF->SBUF DMA per head-pair shifts the recip to partitions 0:64,
    then the PSUM eviction multiply (DVE) divides. Odd heads write oT[64:128]
    directly -- DVE ops may write a different 64-aligned partition base than
    they read as long as both INPUTS share a base (HW-verified).
  - final projection in transposed layout outT[E, Sq]: lhsT=wo tiles,
    rhs=oT tiles, bias bo added by ScalarE at eviction (per-partition).
    The host transposes outT back when assembling the full output.
  - emission interleaves, per 512-row chunk c and per et in 0..7:
    QT(c+1) group et (8 MMs) -> attention(c) pair et (4 MMs) ->
    final(c-1) group et (8 MMs), keeping the PE queue dense so ScalarE/DVE
    latency never drains it. ~20 dummy warmup MMs at t=0 bridge the initial
    weight DMA so the PE HAM clock-gate reaches 8/8 before real work.
"""

import os
from contextlib import ExitStack

import numpy as np
import ml_dtypes

import concourse.bass as bass
import concourse.tile as tile
from concourse import bacc, mybir
from concourse.bass_utils import run_bass_kernel_spmd

N_CORES = 8
SQ = 4096
SK = 77
SKP = 80  # padded free size for phase-0 kT psum tiles
E = 1024
C = 768
CA = C + 1  # y augmented with a ones-row (folds bk/bv)
H = 16
D = 64
CHUNK = 512
NCHUNK = SQ // CHUNK  # 8
ET = E // 128  # 8 e-tiles
F32 = mybir.dt.float32
BF16 = mybir.dt.bfloat16
BF = ml_dtypes.bfloat16

N_WARM = 36  # dummy PE warmup matmuls bridging the prologue DMA

_PROGRAM = None


def _build_program():
    nc = bacc.Bacc(
        "TRN2", target_bir_lowering=False, debug=False, num_devices=N_CORES
    )
    dz_d = nc.dram_tensor("dz", [128, 128 + CHUNK], BF16, kind="ExternalInput").ap()
    # xT pre-tiled host-side: [chunk, partition, e-tile, col]; 1MB per chunk.
    xT_d = nc.dram_tensor(
        "xT", [NCHUNK, 128, ET, CHUNK], BF16, kind="ExternalInput"
    ).ap()
    wq_d = nc.dram_tensor("wq", [E, E], BF16, kind="ExternalInput").ap()
    bq_d = nc.dram_tensor("bq", [E], F32, kind="ExternalInput").ap()
    yT_d = nc.dram_tensor("yT", [CA, SK], BF16, kind="ExternalInput").ap()
    wk_d = nc.dram_tensor("wk", [CA, E], BF16, kind="ExternalInput").ap()
    wv_d = nc.dram_tensor("wv", [CA, H * 64], BF16, kind="ExternalInput").ap()
    vones_d = nc.dram_tensor("vones", [SK, H, 64], BF16, kind="ExternalInput").ap()
    # wo pre-arranged for the transposed final: [p, t, eb, col]
    wo_d = nc.dram_tensor("wo", [128, ET, ET, 128], BF16, kind="ExternalInput").ap()
    bo_d = nc.dram_tensor("bo", [E], F32, kind="ExternalInput").ap()
    outT_d = nc.dram_tensor("outT", [ET, 128, SQ], F32, kind="ExternalOutput").ap()

    with tile.TileContext(nc) as tc, ExitStack() as ctx:
        consts = ctx.enter_context(tc.tile_pool(name="consts", bufs=1))
        dz_sb = consts.tile([128, 128 + CHUNK], BF16)
        wq_sb = consts.tile([128, ET, E], BF16)
        wo_sb = consts.tile([128, ET, ET, 128], BF16)
        kT_sb = consts.tile([128, ET, SKP], BF16)
        v_sb = consts.tile([SK, H, 128], BF16)
        bq_sb = consts.tile([128, ET], F32)
        bo_sb = consts.tile([128, ET], F32)

        # Prologue DMA order = consumption order: warmup tile, xT(0), wq
        # (per-et column blocks so QT(0) group 0 starts after ~1.25MB), then
        # the phase-0 inputs, then wo (first needed by final(0) in chunk 1).
        nc.sync.dma_start(dz_sb[:], dz_d)

        xT_pool = ctx.enter_context(tc.tile_pool(name="xT", bufs=2))
        qT_pool = ctx.enter_context(tc.tile_pool(name="qT", bufs=2))
        oT_pool = ctx.enter_context(tc.tile_pool(name="oT", bufs=2))
        exps_pool = ctx.enter_context(tc.tile_pool(name="exps", bufs=4))
        rbhi_pool = ctx.enter_context(tc.tile_pool(name="rbhi", bufs=3))
        rblo_pool = ctx.enter_context(tc.tile_pool(name="rblo", bufs=3))
        outs_pool = ctx.enter_context(tc.tile_pool(name="outs", bufs=3))
        ps_qf = ctx.enter_context(tc.tile_pool(name="ps_qf", bufs=3, space="PSUM"))
        ps_s = ctx.enter_context(tc.tile_pool(name="ps_s", bufs=2, space="PSUM"))
        ps_av = ctx.enter_context(tc.tile_pool(name="ps_av", bufs=3, space="PSUM"))

        def load_xT(c):
            xT_sb = xT_pool.tile([128, ET, CHUNK], BF16, tag="xT")
            nc.sync.dma_start(xT_sb[:], xT_d[c])
            return xT_sb

        xT_cur = load_xT(0)
        wq_r = wq_d.rearrange("(t p) n -> p t n", p=128)
        for et in range(ET):
            nc.sync.dma_start(
                wq_sb[:, :, et * 128 : (et + 1) * 128],
                wq_r[:, :, et * 128 : (et + 1) * 128],
            )
        nc.sync.dma_start(bq_sb[:], bq_d.rearrange("(t p) -> p t", p=128))

        # PE warmup: garbage-free dummy accumulations on the zero tile.
        ps_warm = ps_av.tile([128, CHUNK], F32, tag="psav", name="warm")
        for i in range(N_WARM):
            nc.tensor.matmul(
                ps_warm[:],
                dz_sb[:, 0:128],
                dz_sb[:, 128 : 128 + CHUNK],
                start=(i == 0),
                stop=(i == N_WARM - 1),
            )

        # Phase-0 input DMAs (consumed after QT(0) below).
        ph0 = ctx.enter_context(tc.tile_pool(name="ph0", bufs=1))
        CTT = (CA + 127) // 128  # 7 contraction tiles; last has 65 rows
        yT_sb = ph0.tile([128, CTT, SK], BF16)
        wk_sb = ph0.tile([128, CTT, E], BF16)
        wv_sb = ph0.tile([128, CTT, H * 64], BF16)
        for t in range(CTT):
            rows = min(128, CA - t * 128)
            nc.sync.dma_start(yT_sb[0:rows, t, :], yT_d[t * 128 : t * 128 + rows, :])
            nc.sync.dma_start(wk_sb[0:rows, t, :], wk_d[t * 128 : t * 128 + rows, :])
            nc.sync.dma_start(wv_sb[0:rows, t, :], wv_d[t * 128 : t * 128 + rows, :])
        nc.sync.dma_start(v_sb[:, :, 0:64], vones_d)
        for t in range(ET):
            nc.sync.dma_start(wo_sb[:, t, :, :], wo_d[:, t, :, :])
        nc.sync.dma_start(bo_sb[:], bo_d.rearrange("(t p) -> p t", p=128))

        def emit_qt_group(xT_sb, qT_sb, et):
            ps = ps_qf.tile([128, CHUNK], F32, tag="psqf")
            for t in range(ET):
                nc.tensor.matmul(
                    ps[:],
                    wq_sb[:, t, et * 128 : (et + 1) * 128],
                    xT_sb[:, t, :],
                    start=(t == 0),
                    stop=(t == ET - 1),
                )
            nc.scalar.activation(
                qT_sb[:, et, :],
                ps[:],
                mybir.ActivationFunctionType.Identity,
                bias=bq_sb[:, et : et + 1],
            )

        # QT(0): 8 groups, standalone (phase 0 weights still streaming in).
        qT_cur = qT_pool.tile([128, ET, CHUNK], BF16, tag="qT")
        for et in range(ET):
            emit_qt_group(xT_cur, qT_cur, et)

        # Phase 0: kT[E, Sk] and V[Sk, H*128] (biases via y ones-row).
        for et in range(ET):
            psk = ps_s.tile([128, SKP], F32, tag="pss", name="psk")
            for t in range(CTT):
                rows = min(128, CA - t * 128)
                nc.tensor.matmul(
                    psk[:, 0:SK],
                    wk_sb[0:rows, t, et * 128 : (et + 1) * 128],
                    yT_sb[0:rows, t, :],
                    start=(t == 0),
                    stop=(t == CTT - 1),
                )
            with nc.allow_low_precision(reason="bf16 kT operand"):
                nc.vector.tensor_copy(kT_sb[:, et, 0:SK], psk[:, 0:SK])
        for g in range(2):
            psv = ps_av.tile([SK, CHUNK], F32, tag="psav", name="psv")
            for t in range(CTT):
                rows = min(128, CA - t * 128)
                nc.tensor.matmul(
                    psv[:],
                    yT_sb[0:rows, t, :],
                    wv_sb[0:rows, t, g * CHUNK : (g + 1) * CHUNK],
                    start=(t == 0),
                    stop=(t == CTT - 1),
                )
            for hh in range(8):
                h = g * 8 + hh
                with nc.allow_low_precision(reason="bf16 V operand"):
                    nc.vector.tensor_copy(
                        v_sb[:, h, 64:128], psv[:, hh * 64 : (hh + 1) * 64]
                    )

        def emit_attn_pair(qT_sb, oT_sb, et):
            hA, hB = 2 * et, 2 * et + 1
            psa = ps_s.tile([SK, CHUNK], F32, tag="pss")
            psb = ps_s.tile([SK, CHUNK], F32, tag="pss")
            nc.tensor.matmul(
                psa[:], kT_sb[0:64, et, 0:SK], qT_sb[0:64, et, :],
                start=True, stop=True,
            )
            nc.tensor.matmul(
                psb[:], kT_sb[64:128, et, 0:SK], qT_sb[64:128, et, :],
                start=True, stop=True,
            )
            exa = exps_pool.tile([SK, CHUNK], BF16, tag="exps")
            exb = exps_pool.tile([SK, CHUNK], BF16, tag="exps")
            nc.scalar.activation(exa[:], psa[:], mybir.ActivationFunctionType.Exp)
            nc.scalar.activation(exb[:], psb[:], mybir.ActivationFunctionType.Exp)
            pava = ps_av.tile([128, CHUNK], F32, tag="psav")
            pavb = ps_av.tile([128, CHUNK], F32, tag="psav")
            nc.tensor.matmul(
                pava[:], v_sb[:, hA, :], exa[:], start=True, stop=True
            )
            nc.tensor.matmul(
                pavb[:], v_sb[:, hB, :], exb[:], start=True, stop=True
            )
            # PSUM layout per head is [den 0:64 | att 64:128] (lhsT=[ones|V]).
            # fast-reciprocal only works at base partition 0, so recip the den
            # halves there, DMA-shift the pair up to partitions 64:128, and
            # evict with a multiply whose inputs are both base-64 (the dst
            # may sit at either 64-aligned base; HW-verified).
            rblo = rblo_pool.tile([128, 2 * CHUNK], F32, tag="rblo")
            rbhi = rbhi_pool.tile([128, 2 * CHUNK], F32, tag="rbhi")
            nc.vector.reciprocal_approx_fast(rblo[0:64, 0:CHUNK], pava[0:64, :])
            nc.vector.reciprocal_approx_fast(
                rblo[0:64, CHUNK : 2 * CHUNK], pavb[0:64, :]
            )
            # Move the recip pair to partitions 64:128 (cross-base writes are
            # legal for single-input ops); ~4x cheaper end-to-end than an
            # SBUF->SBUF DMA whose completion receipt costs ~2us. Alternate
            # the engine per pair to balance ScalarE/VectorE occupancy.
            if et % 2 == 0:
                nc.scalar.activation(
                    rbhi[64:128, :], rblo[0:64, :],
                    mybir.ActivationFunctionType.Identity,
                )
            else:
                nc.vector.tensor_copy(rbhi[64:128, :], rblo[0:64, :])
            with nc.allow_low_precision(reason="bf16 attention output"):
                nc.vector.tensor_tensor(
                    oT_sb[0:64, et, :], pava[64:128, :], rbhi[64:128, 0:CHUNK],
                    mybir.AluOpType.mult,
                )
                nc.vector.tensor_tensor(
                    oT_sb[64:128, et, :], pavb[64:128, :],
                    rbhi[64:128, CHUNK : 2 * CHUNK], mybir.AluOpType.mult,
                )

        def emit_final_group(c, oT_sb, eb):
            ps = ps_qf.tile([128, CHUNK], F32, tag="psqf", name="psf")
            for t in range(ET):
                nc.tensor.matmul(
                    ps[:],
                    wo_sb[:, t, eb, :],
                    oT_sb[:, t, :],
                    start=(t == 0),
                    stop=(t == ET - 1),
                )
            o_sb = outs_pool.tile([128, CHUNK], F32, tag="osb")
            nc.scalar.activation(
                o_sb[:],
                ps[:],
                mybir.ActivationFunctionType.Identity,
                bias=bo_sb[:, eb : eb + 1],
            )
            nc.sync.dma_start(
                outT_d[eb, :, c * CHUNK : (c + 1) * CHUNK], o_sb[:]
            )

        # Main software pipeline over chunks:
        #   body(c): per et: QT(c+1) group et | attn(c) pair et |
        #            final(c-1) group et
        prev = None  # (c, oT_sb) awaiting final projection
        for c in range(NCHUNK):
            xT_sb = xT_cur
            qT_sb = qT_cur
            if c + 1 < NCHUNK:
                xT_cur = load_xT(c + 1)
                qT_cur = qT_pool.tile([128, ET, CHUNK], BF16, tag="qT")
            oT_sb = oT_pool.tile([128, ET, CHUNK], BF16, tag="oT")
            for et in range(ET):
                if c + 1 < NCHUNK:
                    emit_qt_group(xT_cur, qT_cur, et)
                emit_attn_pair(qT_sb, oT_sb, et)
                if prev is not None:
                    emit_final_group(prev[0], prev[1], et)
            prev = (c, oT_sb)
        for eb in range(ET):
            emit_final_group(prev[0], prev[1], eb)

    nc.compile()
    return nc


def _get_program():
    global _PROGRAM
    if _PROGRAM is None:
        _PROGRAM = _build_program()
    return _PROGRAM


def kernel(x, y, wq, bq, wk, bk, wv, bv, wo, bo):
    x = np.asarray(x, dtype=np.float32)
    y = np.asarray(y, dtype=np.float32)
    wq = np.asarray(wq, dtype=np.float32)
    bq = np.asarray(bq, dtype=np.float32)
    wk = np.asarray(wk, dtype=np.float32)
    bk = np.asarray(bk, dtype=np.float32)
    wv = np.asarray(wv, dtype=np.float32)
    bv = np.asarray(bv, dtype=np.float32)
    wo = np.asarray(wo, dtype=np.float32)
    bo = np.asarray(bo, dtype=np.float32)

    scale = np.float32(1.0 / np.sqrt(D))
    wq_s = (wq * scale).astype(BF)
    bq_s = (bq * scale).astype(np.float32)

    # Fold k/v biases via an appended ones-row on y.
    wk_a = np.concatenate([wk, bk[None, :]], axis=0).astype(BF)
    wv_a = np.concatenate([wv, bv[None, :]], axis=0).astype(BF)
    wo_r = np.ascontiguousarray(
        wo.reshape(ET, 128, ET, 128).transpose(1, 0, 2, 3)
    ).astype(BF)
    vones = np.ones((SK, H, 64), dtype=BF)
    dz = np.zeros((128, 128 + CHUNK), dtype=BF)

    nc = _get_program()
    in_maps = []
    for b in range(N_CORES):
        xT = np.ascontiguousarray(
            x[b].T.reshape(ET, 128, NCHUNK, CHUNK).transpose(2, 1, 0, 3)
        ).astype(BF)
        yT = np.concatenate(
            [y[b].T, np.ones((1, SK), dtype=np.float32)], axis=0
        ).astype(BF)
        in_maps.append(
            {
                "dz": dz,
                "xT": xT,
                "wq": wq_s,
                "bq": bq_s,
                "yT": yT,
                "wk": wk_a,
                "wv": wv_a,
                "vones": vones,
                "wo": wo_r,
                "bo": bo,
            }
        )

    trace = bool(int(os.environ.get("KERNEL_TRACE", "0")))
    kwargs = {}
    if trace:
        kwargs = {"trace": True, "tmpdir": os.environ.get("KERNEL_TRACE_DIR")}
    try:
        res = run_bass_kernel_spmd(nc, in_maps, list(range(N_CORES)), **kwargs)
    except Exception:
        # The axon-tunneled devices occasionally report a transient
        # NRT_EXEC_UNIT_UNRECOVERABLE; a retry on the same executable has
        # been observed to succeed.
        res = run_bass_kernel_spmd(nc, in_maps, list(range(N_CORES)), **kwargs)
    if trace:
        kernel.last_exec_time_ns = res.exec_time_ns
        kernel.last_results = res
    out = np.stack(
        [
            np.ascontiguousarray(
                res.results[b]["outT"].reshape(E, SQ).T
            )
            for b in range(N_CORES)
        ]
    )
    return out
